# revision 21
# baseline (speedup 1.0000x reference)
"""AGDN (2-layer, K=3 hop) GNN message passing on 8 TRN2 NeuronCores.

v3 strategy (node-sharded, replicated fp16 feature table):
  - Nodes padded to 50176 = 8 * 6272; core c owns dst rows [c*6272, (c+1)*6272).
  - One edge schedule shared by all 6 hops: edges grouped per (128-dst window,
    src parity) into 128-edge chunks; per chunk a one-hot S (DVE is_equal vs
    iota) selects dst lanes and a PE matmul S^T @ G accumulates into the
    window's PSUM tile.
  - G rows come from SWDGE dma_gather of 256-byte fp16 node PAIRS out of the
    replicated HBM table (int16 pair indices resident in SBUF, loaded once);
    the parity key picks the 64-wide half of each gathered row at matmul time.
  - fp16 everywhere on the hop path with per-hop 16^-k storage scaling
    (hop values grow ~16x/hop and would overflow fp16 unscaled); the scale is
    undone on the attention scores and folded into the softmax weights.
  - Between hops an 8-rank fp16 AllGather rebuilds the replicated table.
  - Dense layers, hop-attention softmax and ELU run locally per shard.
"""

import numpy as np
import ml_dtypes

import concourse.bacc as bacc
import concourse.mybir as mybir
import concourse.tile as tile
from concourse.bass_utils import run_bass_kernel_spmd

N = 50000
E = 800000
DIN = 128
D = 64
NCORES = 8
P = 128
WPC = 49                  # 128-dst windows per core
NB = WPC * P              # 6272 nodes per core
NPAD = NCORES * NB        # 50176
NG = WPC * 2              # (window, parity) groups per core
PIECE = 64                # chunks per gather piece
SBATCH = 8                # chunks per DVE S-build op
SLOPE = 0.2

F32 = mybir.dt.float32
F16 = mybir.dt.float16
I16 = mybir.dt.int16
NF16 = np.float16


def _schedule(edge_index):
    """One shared schedule for all hops.

    Returns (CH, meta, pieces, idx_all, dstw_all):
      meta:   per chunk (w, key, first, last)
      pieces: list of (c0, npc)
      idx_all[c]:  [P, CH*8] int16 wrapped pair-row gather indices
      dstw_all[c]: [P, CH] fp16 dst lane in window (-1 pad)
    """
    src = np.ascontiguousarray(edge_index[0]).astype(np.int64)
    dst = np.ascontiguousarray(edge_index[1]).astype(np.int64)
    core = dst // NB
    wloc = (dst % NB) >> 7
    dstw = dst & 127
    sc = src // NB
    sr = src % NB
    m_src = sc * NB + (sr & 127) * WPC + (sr >> 7)   # device node id
    key = m_src & 1
    pair = m_src >> 1                                 # int16-safe (< 25088)

    g_in_core = wloc * 2 + key
    gkey = core * NG + g_in_core
    order = np.argsort(gkey, kind="stable")
    gk_sorted = gkey[order]
    run_start = np.searchsorted(gk_sorted, gk_sorted)
    pos_in_group = np.arange(E) - run_start
    inv = np.empty(E, dtype=np.int64)
    inv[order] = np.arange(E)
    pos_of_edge = pos_in_group[inv]

    cnt = np.bincount(gkey, minlength=NCORES * NG).reshape(NCORES, NG)
    gmax = cnt.max(axis=0)
    nchunks = np.ceil(gmax / P).astype(np.int64)
    for w in range(WPC):
        if nchunks[2 * w] + nchunks[2 * w + 1] == 0:
            nchunks[2 * w] = 1
    CH = int(nchunks.sum())

    gfirst = np.zeros(NG, dtype=np.int64)
    meta = []
    c0 = 0
    for w in range(WPC):
        wtot = int(nchunks[2 * w] + nchunks[2 * w + 1])
        seen = 0
        for k in (0, 1):
            gfirst[2 * w + k] = c0
            for _ in range(int(nchunks[2 * w + k])):
                meta.append((w, k, seen == 0, seen == wtot - 1))
                seen += 1
                c0 += 1
    assert c0 == CH

    pieces = []
    p0 = 0
    while p0 < CH:
        pieces.append((p0, min(PIECE, CH - p0)))
        p0 += PIECE

    idx_all, dstw_all = [], []
    nslots = CH * P
    for c in range(NCORES):
        mask = core == c
        gl = g_in_core[mask]
        pos = pos_of_edge[mask]
        ch = gfirst[gl] + (pos >> 7)
        slot = ch * P + (pos & 127)
        iarr = np.zeros(nslots, dtype=np.int16)
        darr = np.full(nslots, -1.0, dtype=np.float32)
        iarr[slot] = pair[mask].astype(np.int16)
        darr[slot] = dstw[mask].astype(np.float32)
        wrapped = np.tile(iarr.reshape(CH * 8, 16).T, (8, 1))
        idx_all.append(np.ascontiguousarray(wrapped))
        dstw_all.append(np.ascontiguousarray(
            darr.reshape(CH, P).T.astype(NF16)))
    return CH, meta, pieces, idx_all, dstw_all


def _build(CH, meta, pieces):
    nc = bacc.Bacc("TRN2", target_bir_lowering=False, debug=False,
                   num_devices=NCORES, num_swdge_queues=4)

    with tile.TileContext(nc) as tc:
        xT = nc.dram_tensor("xT", [WPC, P, P], F16, kind="ExternalInput")
        idxp = nc.dram_tensor("idxp", [P, CH * 8], I16, kind="ExternalInput")
        dstwp = nc.dram_tensor("dstwp", [P, CH], F16, kind="ExternalInput")
        iotap = nc.dram_tensor("iotap", [P, P], F16, kind="ExternalInput")
        identp = nc.dram_tensor("identp", [P, P], F16, kind="ExternalInput")
        w1p = nc.dram_tensor("w1p", [P, P], F16, kind="ExternalInput")
        w2tp = nc.dram_tensor("w2tp", [D, D], F16, kind="ExternalInput")
        attp = nc.dram_tensor("attp", [P, 4 * D], F32, kind="ExternalInput")
        scalep = nc.dram_tensor("scalep", [P, 4], F32, kind="ExternalInput")
        biasp = nc.dram_tensor("biasp", [P, 2 * D], F32, kind="ExternalInput")
        outp = nc.dram_tensor("out", [NB, D], F32, kind="ExternalOutput")

        ccs = [nc.dram_tensor(f"cc{i}", [NB, D], F16) for i in range(3)]
        tabs = [nc.dram_tensor(f"gtab{i}", [NPAD, D], F16,
                               addr_space="Shared") for i in range(3)]

        with tc.tile_pool(name="const", bufs=1) as pconst, \
             tc.tile_pool(name="big", bufs=1) as pbig, \
             tc.tile_pool(name="xt", bufs=3) as pxt, \
             tc.tile_pool(name="outl", bufs=1) as pout, \
             tc.tile_pool(name="g2", bufs=2) as pg2, \
             tc.tile_pool(name="sseg", bufs=4) as pseg, \
             tc.tile_pool(name="att", bufs=1) as patt, \
             tc.tile_pool(name="psum", bufs=4, space="PSUM") as pps, \
             tc.tile_pool(name="psumd", bufs=2, space="PSUM") as ppsd:

            iota_sb = pconst.tile([P, P], F16)
            nc.sync.dma_start(out=iota_sb[:], in_=iotap[:])
            ident_sb = pconst.tile([P, P], F16)
            nc.sync.dma_start(out=ident_sb[:], in_=identp[:])
            w1_sb = pconst.tile([P, P], F16)
            nc.sync.dma_start(out=w1_sb[:], in_=w1p[:])
            w2t_sb = pconst.tile([D, D], F16)
            nc.sync.dma_start(out=w2t_sb[:], in_=w2tp[:])
            att_sb = pconst.tile([P, 4 * D], F32)
            nc.sync.dma_start(out=att_sb[:], in_=attp[:])
            hscale_sb = pconst.tile([P, 4], F32)
            nc.sync.dma_start(out=hscale_sb[:], in_=scalep[:])
            bias_sb = pconst.tile([P, 2 * D], F32)
            nc.sync.dma_start(out=bias_sb[:], in_=biasp[:])
            idx_sb = pconst.tile([P, CH * 8], I16)
            nc.sync.dma_start(out=idx_sb[:], in_=idxp[:])
            dstw_sb = pconst.tile([P, CH], F16)
            nc.sync.dma_start(out=dstw_sb[:], in_=dstwp[:])

            stack1 = pbig.tile([P, 4, WPC, D], F16)
            stack2 = pbig.tile([P, 4, WPC, D], F16)
            res1_sb = pbig.tile([P, WPC, D], F16)
            tmp_sb = pbig.tile([P, WPC, D], F32)
            acc_sb = pbig.tile([P, WPC, D], F32)
            g_sb = pbig.tile([P, WPC, D], F16)
            gT_sb = pbig.tile([D, WPC, P], F16)

            qctr = [0]

            def exchange(stack, k, t):
                """AllGather stack[:, k] -> tabs[t] on every core."""
                nc.sync.dma_start(
                    out=ccs[t][:].rearrange("(p w) d -> p w d", p=P),
                    in_=stack[:, k, :, :],
                )
                nc.gpsimd.collective_compute(
                    "AllGather", mybir.AluOpType.bypass,
                    replica_groups=[list(range(NCORES))],
                    ins=[ccs[t][:].opt()],
                    outs=[tabs[t][:].opt()],
                )

            def hop(t, stack, k):
                """stack[:, k] = segment_sum over edges of tabs[t][src]/16."""
                base = tabs[t][:].rearrange("(a b) d -> a (b d)", b=2)
                pw = None
                for (c0, npc) in pieces:
                    g2 = pg2.tile([P, PIECE, 2 * D], F16, tag="g2", name="g2")
                    nc.gpsimd.dma_gather(
                        g2[:, 0:npc, :], base,
                        idx_sb[:, c0 * 8:(c0 + npc) * 8],
                        npc * P, npc * P, 2 * D,
                        single_packet=False, queue_num=qctr[0] % 4,
                    )
                    qctr[0] += 1
                    sb_tiles = {}
                    for b0 in range(0, npc, SBATCH):
                        b1 = min(b0 + SBATCH, npc)
                        st = pseg.tile([P, SBATCH * P], F16, tag="sseg",
                                       name="st")
                        nc.vector.tensor_tensor(
                            out=st[:, 0:(b1 - b0) * P].rearrange(
                                "p (c q) -> p c q", q=P),
                            in0=dstw_sb[:, c0 + b0:c0 + b1].to_broadcast(
                                [P, b1 - b0, P]),
                            in1=iota_sb[:].unsqueeze(1).broadcast_to(
                                [P, b1 - b0, P]),
                            op=mybir.AluOpType.is_equal,
                        )
                        sb_tiles[b0] = st
                    for c in range(c0, c0 + npc):
                        w, key, first, last = meta[c]
                        if first:
                            pw = pps.tile([P, D], F32, tag="pwin", name="pw")
                        lc = c - c0
                        st = sb_tiles[(lc // SBATCH) * SBATCH]
                        jj = lc % SBATCH
                        nc.tensor.matmul(
                            out=pw[:], lhsT=st[:, jj * P:(jj + 1) * P],
                            rhs=g2[:, lc, key * D:(key + 1) * D],
                            start=first, stop=last,
                        )
                        if last:
                            # hop values stored scaled by 16^-k (fp16 range)
                            nc.scalar.activation(
                                out=stack[:, k, w, :], in_=pw[:],
                                func=mybir.ActivationFunctionType.Copy,
                                scale=1.0 / 16.0)

            def attention_early(stack, bias_col, sm):
                a_hop = att_sb[:, (2 * bias_col + 1) * D:(2 * bias_col + 2) * D]
                a_h0 = att_sb[:, (2 * bias_col) * D:(2 * bias_col + 1) * D]
                scf = sm[:, :, 0:4]
                sc0 = sm[:, :, 4:5]
                for k in range(3):
                    nc.vector.tensor_tensor(
                        out=tmp_sb[:], in0=stack[:, k, :, :],
                        in1=a_hop.unsqueeze(1).broadcast_to([P, WPC, D]),
                        op=mybir.AluOpType.mult)
                    nc.vector.reduce_sum(out=scf[:, :, k:k + 1], in_=tmp_sb[:],
                                         axis=mybir.AxisListType.X)
                nc.vector.tensor_tensor(
                    out=tmp_sb[:], in0=stack[:, 0, :, :],
                    in1=a_h0.unsqueeze(1).broadcast_to([P, WPC, D]),
                    op=mybir.AluOpType.mult)
                nc.vector.reduce_sum(out=sc0, in_=tmp_sb[:],
                                     axis=mybir.AxisListType.X)

            def attention(stack, res_ap, bias_col, out_tile, sm):
                a_hop = att_sb[:, (2 * bias_col + 1) * D:(2 * bias_col + 2) * D]
                scf = sm[:, :, 0:4]
                scores = sm[:, :, 0:4]
                sc0 = sm[:, :, 4:5]
                mx = sm[:, :, 5:6]
                ssum = sm[:, :, 6:7]
                rec = sm[:, :, 7:8]
                nc.vector.tensor_tensor(
                    out=tmp_sb[:], in0=stack[:, 3, :, :],
                    in1=a_hop.unsqueeze(1).broadcast_to([P, WPC, D]),
                    op=mybir.AluOpType.mult)
                nc.vector.reduce_sum(out=scf[:, :, 3:4], in_=tmp_sb[:],
                                     axis=mybir.AxisListType.X)
                # undo the per-hop 16^-k storage scale on the raw scores
                nc.vector.tensor_tensor(
                    out=scf, in0=scf,
                    in1=hscale_sb[:].unsqueeze(1).broadcast_to([P, WPC, 4]),
                    op=mybir.AluOpType.mult)
                nc.vector.tensor_tensor(
                    out=scores, in0=scf, in1=sc0.broadcast_to([P, WPC, 4]),
                    op=mybir.AluOpType.add)
                nc.vector.scalar_tensor_tensor(
                    out=scores, in0=scores, scalar=SLOPE, in1=scores,
                    op0=mybir.AluOpType.mult, op1=mybir.AluOpType.max)
                nc.vector.reduce_max(out=mx, in_=scores,
                                     axis=mybir.AxisListType.X)
                nc.vector.tensor_tensor(
                    out=scores, in0=scores, in1=mx.broadcast_to([P, WPC, 4]),
                    op=mybir.AluOpType.subtract)
                nc.scalar.activation(out=scores, in_=scores,
                                     func=mybir.ActivationFunctionType.Exp)
                nc.vector.reduce_sum(out=ssum, in_=scores,
                                     axis=mybir.AxisListType.X)
                nc.vector.reciprocal(out=rec, in_=ssum)
                nc.vector.tensor_tensor(
                    out=scores, in0=scores, in1=rec.broadcast_to([P, WPC, 4]),
                    op=mybir.AluOpType.mult)
                # fold the 16^k unscale into the softmax weights
                nc.vector.tensor_tensor(
                    out=scores, in0=scores,
                    in1=hscale_sb[:].unsqueeze(1).broadcast_to([P, WPC, 4]),
                    op=mybir.AluOpType.mult)
                nc.vector.tensor_tensor(
                    out=acc_sb[:], in0=stack[:, 0, :, :],
                    in1=scores[:, :, 0:1].broadcast_to([P, WPC, D]),
                    op=mybir.AluOpType.mult)
                for k in range(1, 4):
                    nc.vector.tensor_tensor(
                        out=tmp_sb[:], in0=stack[:, k, :, :],
                        in1=scores[:, :, k:k + 1].broadcast_to([P, WPC, D]),
                        op=mybir.AluOpType.mult)
                    nc.vector.tensor_tensor(out=acc_sb[:], in0=acc_sb[:],
                                            in1=tmp_sb[:],
                                            op=mybir.AluOpType.add)
                nc.vector.tensor_tensor(out=acc_sb[:], in0=acc_sb[:],
                                        in1=res_ap, op=mybir.AluOpType.add)
                b = bias_sb[:, bias_col * D:(bias_col + 1) * D]
                nc.vector.tensor_tensor(
                    out=out_tile, in0=acc_sb[:],
                    in1=b.unsqueeze(1).broadcast_to([P, WPC, D]),
                    op=mybir.AluOpType.add)

            # ---------------- layer 1 ----------------
            for t in range(WPC):
                xtile = pxt.tile([P, P], F16, tag="xt", name="xtile")
                nc.sync.dma_start(out=xtile[:], in_=xT[t, :, :])
                ps = ppsd.tile([P, P], F32, tag="pd")
                nc.tensor.matmul(out=ps[:], lhsT=xtile[:], rhs=w1_sb[:],
                                 start=True, stop=True)
                nc.scalar.copy(out=stack1[:, 0, t, :], in_=ps[:, 0:D])
                nc.vector.tensor_copy(out=res1_sb[:, t, :], in_=ps[:, D:P])

            exchange(stack1, 0, 0)
            hop(0, stack1, 1)
            exchange(stack1, 1, 1)
            hop(1, stack1, 2)
            exchange(stack1, 2, 2)
            sm1 = patt.tile([P, WPC, 8], F32, tag="attsm", name="sm1")
            attention_early(stack1, 0, sm1)
            hop(2, stack1, 3)

            attention(stack1, res1_sb[:], 0, acc_sb[:], sm1)
            # ELU -> g_sb (fp16)
            gm = tmp_sb[:]
            nc.vector.tensor_scalar_min(out=gm, in0=acc_sb[:], scalar1=0.0)
            nc.scalar.activation(out=gm, in_=gm,
                                 func=mybir.ActivationFunctionType.Exp)
            nc.vector.tensor_scalar_max(out=acc_sb[:], in0=acc_sb[:],
                                        scalar1=0.0)
            nc.vector.scalar_tensor_tensor(
                out=g_sb[:], in0=gm, scalar=-1.0, in1=acc_sb[:],
                op0=mybir.AluOpType.add, op1=mybir.AluOpType.add)

            # ---------------- layer 2 ----------------
            for t in range(WPC):
                pst = ppsd.tile([D, P], F16, tag="pd", name="pst")
                nc.tensor.transpose(out=pst[:], in_=g_sb[:, t, :],
                                    identity=ident_sb[:])
                nc.vector.tensor_copy(out=gT_sb[:, t, :], in_=pst[:])
            for t in range(WPC):
                ps = ppsd.tile([P, D], F32, tag="pd", name="ps2")
                nc.tensor.matmul(out=ps[:], lhsT=gT_sb[:, t, :],
                                 rhs=w2t_sb[:], start=True, stop=True)
                nc.scalar.copy(out=stack2[:, 0, t, :], in_=ps[:])
            exchange(stack2, 0, 0)
            hop(0, stack2, 1)
            exchange(stack2, 1, 1)
            hop(1, stack2, 2)
            exchange(stack2, 2, 2)
            sm2 = patt.tile([P, WPC, 8], F32, tag="attsm", name="sm2")
            attention_early(stack2, 1, sm2)
            hop(2, stack2, 3)

            out2_sb = pout.tile([P, WPC, D], F32, tag="outl", name="out2_sb")
            attention(stack2, stack2[:, 0, :, :], 1, out2_sb[:], sm2)
            nc.sync.dma_start(
                out=outp[:].rearrange("(p w) d -> p w d", p=P),
                in_=out2_sb[:],
            )
    nc.compile()
    return nc


_CACHE = {}
_last_in_maps = None


def kernel(**inputs):
    x = np.asarray(inputs["x"], dtype=np.float32)
    edge_index = np.asarray(inputs["edge_index"])
    W1 = np.asarray(inputs["W1"], dtype=np.float32)
    att1 = np.asarray(inputs["att1"], dtype=np.float32)
    bias1 = np.asarray(inputs["bias1"], dtype=np.float32)
    resW1 = np.asarray(inputs["resW1"], dtype=np.float32)
    W2 = np.asarray(inputs["W2"], dtype=np.float32)
    att2 = np.asarray(inputs["att2"], dtype=np.float32)
    bias2 = np.asarray(inputs["bias2"], dtype=np.float32)

    CH, meta, pieces, idx_all, dstw_all = _schedule(edge_index)

    key = ("v3", CH, len(pieces))
    if key not in _CACHE:
        _CACHE[key] = _build(CH, meta, pieces)
    nc = _CACHE[key]

    xpad = np.zeros((NPAD, DIN), dtype=np.float32)
    xpad[:N] = x
    iota_np = np.tile(np.arange(P, dtype=np.float32)[None, :],
                      (P, 1)).astype(NF16)
    ident_np = np.eye(P, dtype=np.float32).astype(NF16)
    att_np = np.concatenate([
        np.tile(att1[0, 0, :D][None, :], (P, 1)),
        np.tile(att1[0, 0, D:][None, :], (P, 1)),
        np.tile(att2[0, 0, :D][None, :], (P, 1)),
        np.tile(att2[0, 0, D:][None, :], (P, 1)),
    ], axis=1).astype(np.float32)
    bias_np = np.concatenate([
        np.tile(bias1[None, :], (P, 1)),
        np.tile(bias2[None, :], (P, 1)),
    ], axis=1).astype(np.float32)
    scale_np = np.tile(np.array([1.0, 16.0, 256.0, 4096.0],
                                dtype=np.float32)[None, :], (P, 1))
    w1_np = np.concatenate([W1.T, resW1.T], axis=1).astype(NF16)  # [128, 128]
    w2t_np = np.ascontiguousarray(W2.T).astype(NF16)

    jj = np.arange(NB)
    real_in_block = (jj % WPC) * P + (jj // WPC)

    in_maps = []
    for c in range(NCORES):
        xT_c = np.ascontiguousarray(
            xpad[c * NB:(c + 1) * NB].T.reshape(P, WPC, P).transpose(1, 0, 2)
        ).astype(NF16)
        in_maps.append({
            "xT": xT_c,
            "idxp": idx_all[c], "dstwp": dstw_all[c],
            "iotap": iota_np, "identp": ident_np,
            "w1p": w1_np, "w2tp": w2t_np,
            "attp": att_np, "biasp": bias_np, "scalep": scale_np,
        })

    global _last_in_maps
    _last_in_maps = in_maps
    res = run_bass_kernel_spmd(nc, in_maps, core_ids=list(range(NCORES)))
    out = np.empty((NPAD, D), dtype=np.float32)
    for c in range(NCORES):
        out[c * NB + real_in_block] = res.results[c]["out"]
    return out[:N].astype(np.float32)
